# revision 5
# baseline (speedup 1.0000x reference)
"""Trainium2 Bass kernel for nn_BoxDetectionLoss (8-core data parallel).

Math: reference loss = sum_{a,r,c}[ has_match ? coord+conf_loss : conf^2 ] / denom.
A pixel (r,c) can only match a target box t if r==tb[t,0] and c==tb[t,1]
(T=16 boxes per image), so the dense term is just sum sigmoid(conf_ch)^2 over
channels {2,5,8}; the match term is a correction at <=16 pixels x 3 anchors,
computed from 144 gathered elements per image.

Each of the 8 cores handles one batch image:
  - DMA the 3 conf channels (3MB), ACT sigmoid, DVE fused square+reduce.
  - indirect-DMA gather of pol[ch, tb0[t], tb1[t]] for all t, ch.
  - tiny [16,*] vector ops: pred = clip(tb + sigmoid*scale), round-half-even
    via +/- 1.5*2^23 trick, match compare, first-duplicate mask, correction
    contribution (coord + tp*(tp-2c) which equals coord + (c-tp)^2 - c^2).
  - per-partition accumulator [128, cols] -> free-dim reduce -> [128] output.
Host sums the 8x128 partials and divides by denom.
"""

import numpy as np

B, C, H, W = 8, 9, 512, 512
T = 16
N_CORES = 8
CONF_CH = (2, 5, 8)
DENOM = float(B * H * W * 3)
MAGIC = 12582912.0  # 1.5 * 2^23: x+MAGIC-MAGIC rounds to nearest-even int
FSPLIT = 2          # chunks per conf channel
NDENSE = len(CONF_CH) * FSPLIT

TRI_CONST = np.tril(np.ones((T, T), dtype=np.float32), -1)  # [t, t'] = t' < t
CHOFF_CONST = np.broadcast_to(
    (np.arange(C, dtype=np.float32) * (H * W))[None, :], (T, C)
).copy()

_PROG = None


def _build_correction(nc, sp, ACC, bass, mybir, tb, tp, tri, choff, pol,
                      gather=True, bcast=True):
    f32 = mybir.dt.float32
    i32 = mybir.dt.int32
    ALU = mybir.AluOpType
    ACT_F = mybir.ActivationFunctionType

    TB = sp.tile([T, 4], i32)
    nc.sync.dma_start(TB[:], tb[:])
    TP = sp.tile([T, 1], f32)
    nc.sync.dma_start(TP[:], tp[:])
    TRI = sp.tile([T, T], f32)
    nc.sync.dma_start(TRI[:], tri[:])
    CH = sp.tile([T, C], f32)
    nc.sync.dma_start(CH[:], choff[:])
    TBrep = sp.tile([T, 4 * T], i32)  # whole tb replicated per row
    if bcast:
        nc.sync.dma_start(
            TBrep[:], tb.rearrange("t f -> (t f)").partition_broadcast(T)
        )
    else:
        nc.vector.memset(TBrep[:], 0)

    TBf = sp.tile([T, 4], f32)
    nc.vector.tensor_copy(TBf[:], TB[:])
    TBrepf = sp.tile([T, 4 * T], f32)
    nc.vector.tensor_copy(TBrepf[:], TBrep[:])

    # packed coords: p1 = r*512 + c, p2 = r2*512 + c2 (exact in f32)
    p1 = sp.tile([T, 1], f32)
    nc.vector.tensor_scalar(
        out=p1[:], in0=TBf[:, 0:1], scalar1=512.0, scalar2=TBf[:, 1:2],
        op0=ALU.mult, op1=ALU.add,
    )
    p2 = sp.tile([T, 1], f32)
    nc.vector.tensor_scalar(
        out=p2[:], in0=TBf[:, 2:3], scalar1=512.0, scalar2=TBf[:, 3:4],
        op0=ALU.mult, op1=ALU.add,
    )

    # row-layout packed coords of all boxes, from the replicated copy
    rep4 = TBrepf[:].rearrange("p (t f) -> p f t", f=4)
    p1row = sp.tile([T, T], f32)
    nc.vector.tensor_scalar(
        out=p1row[:], in0=rep4[:, 0, :], scalar1=512.0, scalar2=None,
        op0=ALU.mult,
    )
    nc.vector.tensor_tensor(
        out=p1row[:], in0=p1row[:], in1=rep4[:, 1, :], op=ALU.add
    )
    p2row = sp.tile([T, T], f32)
    nc.vector.tensor_scalar(
        out=p2row[:], in0=rep4[:, 2, :], scalar1=512.0, scalar2=None,
        op0=ALU.mult,
    )
    nc.vector.tensor_tensor(
        out=p2row[:], in0=p2row[:], in1=rep4[:, 3, :], op=ALU.add
    )

    # duplicate-box detection: S[t,t'] = (p1 equal) & (p2 equal), t' < t
    S = sp.tile([T, T], f32)
    nc.vector.tensor_scalar(
        out=S[:], in0=p1row[:], scalar1=p1[:], scalar2=None, op0=ALU.is_equal
    )
    S2 = sp.tile([T, T], f32)
    nc.vector.tensor_scalar(
        out=S2[:], in0=p2row[:], scalar1=p2[:], scalar2=None, op0=ALU.is_equal
    )
    nc.vector.tensor_tensor(out=S[:], in0=S[:], in1=S2[:], op=ALU.mult)
    nc.vector.tensor_tensor(out=S[:], in0=S[:], in1=TRI[:], op=ALU.mult)
    dupc = sp.tile([T, 1], f32)
    nc.vector.tensor_reduce(
        out=dupc[:], in_=S[:], axis=mybir.AxisListType.X, op=ALU.add
    )
    keep = sp.tile([T, 1], f32)
    nc.vector.tensor_scalar(
        out=keep[:], in0=dupc[:], scalar1=0.0, scalar2=None, op0=ALU.is_equal
    )

    # gather pol[ch, tb0[t], tb1[t]] for all (t, ch): offsets = ch*H*W + p1
    OFFf = sp.tile([T, C], f32)
    nc.vector.tensor_scalar(
        out=OFFf[:], in0=CH[:], scalar1=p1[:], scalar2=None, op0=ALU.add
    )
    OFFi = sp.tile([T, C], i32)
    nc.vector.tensor_copy(OFFi[:], OFFf[:])
    G = sp.tile([T, C], f32)
    if gather:
        nc.gpsimd.indirect_dma_start(
            out=G[:], out_offset=None,
            in_=pol.rearrange("c h (w a) -> (c h w) a", a=1),
            in_offset=bass.IndirectOffsetOnAxis(ap=OFFi[:], axis=0),
        )
    else:
        nc.vector.memset(G[:], 0.0)
    GS = sp.tile([T, C], f32)
    nc.scalar.activation(GS[:], G[:], ACT_F.Sigmoid)
    # channel ch = 3a + k: k=0 delta_r, k=1 delta_c, k=2 conf
    gs3 = GS[:].rearrange("p (a k) -> p k a", k=3)

    # pred = clip(tb + sigmoid*scale, 0, 511), all 3 anchors at once
    predr = sp.tile([T, 3], f32)
    nc.vector.tensor_scalar(
        out=predr[:], in0=gs3[:, 0, :], scalar1=9.0, scalar2=TBf[:, 0:1],
        op0=ALU.mult, op1=ALU.add,
    )
    nc.vector.tensor_scalar(
        out=predr[:], in0=predr[:], scalar1=511.0, scalar2=0.0,
        op0=ALU.min, op1=ALU.max,
    )
    predc = sp.tile([T, 3], f32)
    nc.vector.tensor_scalar(
        out=predc[:], in0=gs3[:, 1, :], scalar1=16.0, scalar2=TBf[:, 1:2],
        op0=ALU.mult, op1=ALU.add,
    )
    nc.vector.tensor_scalar(
        out=predc[:], in0=predc[:], scalar1=511.0, scalar2=0.0,
        op0=ALU.min, op1=ALU.max,
    )

    # round to nearest-even integer: (x + 1.5*2^23) - 1.5*2^23
    rr = sp.tile([T, 3], f32)
    nc.vector.tensor_scalar(
        out=rr[:], in0=predr[:], scalar1=MAGIC, scalar2=None, op0=ALU.add
    )
    nc.vector.tensor_scalar(
        out=rr[:], in0=rr[:], scalar1=MAGIC, scalar2=None, op0=ALU.subtract
    )
    rc = sp.tile([T, 3], f32)
    nc.vector.tensor_scalar(
        out=rc[:], in0=predc[:], scalar1=MAGIC, scalar2=None, op0=ALU.add
    )
    nc.vector.tensor_scalar(
        out=rc[:], in0=rc[:], scalar1=MAGIC, scalar2=None, op0=ALU.subtract
    )

    # match mask per (t, anchor)
    m = sp.tile([T, 3], f32)
    nc.vector.tensor_scalar(
        out=m[:], in0=rr[:], scalar1=TBf[:, 2:3], scalar2=None, op0=ALU.is_equal
    )
    m2 = sp.tile([T, 3], f32)
    nc.vector.tensor_scalar(
        out=m2[:], in0=rc[:], scalar1=TBf[:, 3:4], scalar2=None, op0=ALU.is_equal
    )
    nc.vector.tensor_tensor(out=m[:], in0=m[:], in1=m2[:], op=ALU.mult)

    # contribution = |predr-tb2| + |predc-tb3| + tp*(tp-2*conf)
    ntb2 = sp.tile([T, 1], f32)
    nc.vector.tensor_scalar(
        out=ntb2[:], in0=TBf[:, 2:3], scalar1=-1.0, scalar2=None, op0=ALU.mult
    )
    ntb3 = sp.tile([T, 1], f32)
    nc.vector.tensor_scalar(
        out=ntb3[:], in0=TBf[:, 3:4], scalar1=-1.0, scalar2=None, op0=ALU.mult
    )
    d1 = sp.tile([T, 3], f32)
    nc.scalar.activation(d1[:], predr[:], ACT_F.Abs, bias=ntb2[:])
    d2 = sp.tile([T, 3], f32)
    nc.scalar.activation(d2[:], predc[:], ACT_F.Abs, bias=ntb3[:])
    nc.vector.tensor_tensor(out=d1[:], in0=d1[:], in1=d2[:], op=ALU.add)
    cf = sp.tile([T, 3], f32)
    nc.vector.tensor_scalar(
        out=cf[:], in0=gs3[:, 2, :], scalar1=-2.0, scalar2=TP[:],
        op0=ALU.mult, op1=ALU.add,
    )
    nc.vector.tensor_scalar(
        out=cf[:], in0=cf[:], scalar1=TP[:], scalar2=None, op0=ALU.mult
    )
    nc.vector.tensor_tensor(out=d1[:], in0=d1[:], in1=cf[:], op=ALU.add)
    # valid = match * keep; corr contribution = valid * d1
    nc.vector.tensor_scalar(
        out=m[:], in0=m[:], scalar1=keep[:], scalar2=None, op0=ALU.mult
    )
    nc.vector.tensor_tensor(out=m[:], in0=m[:], in1=d1[:], op=ALU.mult)
    nc.vector.tensor_reduce(
        out=ACC[0:T, NDENSE : NDENSE + 1], in_=m[:],
        axis=mybir.AxisListType.X, op=ALU.add,
    )


def _build_program(corr=True, gather=True, bcast=True, fsplit=FSPLIT):
    import concourse.bass as bass
    import concourse.tile as tile
    from concourse import bacc, mybir

    f32 = mybir.dt.float32
    i32 = mybir.dt.int32
    ALU = mybir.AluOpType
    ACT_F = mybir.ActivationFunctionType
    free = (H * W // 128) // fsplit
    ndense = len(CONF_CH) * fsplit
    assert ndense == NDENSE or not corr

    nc = bacc.Bacc(
        "TRN2", target_bir_lowering=False, debug=False, num_devices=N_CORES
    )
    pol = nc.dram_tensor("pol", [C, H, W], f32, kind="ExternalInput").ap()
    tb = nc.dram_tensor("tb", [T, 4], i32, kind="ExternalInput").ap()
    tp = nc.dram_tensor("tp", [T, 1], f32, kind="ExternalInput").ap()
    tri = nc.dram_tensor("tri", [T, T], f32, kind="ExternalInput").ap()
    choff = nc.dram_tensor("choff", [T, C], f32, kind="ExternalInput").ap()
    out = nc.dram_tensor("out", [128], f32, kind="ExternalOutput").ap()

    with tile.TileContext(nc) as tc:
        with (
            tc.tile_pool(name="io", bufs=3) as io,
            tc.tile_pool(name="acc", bufs=1) as accp,
            tc.tile_pool(name="small", bufs=1) as sp,
        ):
            ACC = accp.tile([128, ndense + 1], f32)
            nc.vector.memset(ACC[:], 0.0)

            if corr:
                _build_correction(
                    nc, sp, ACC, bass, mybir, tb, tp, tri, choff, pol,
                    gather=gather, bcast=bcast,
                )

            # ---------------- dense path: sum sigmoid(conf_ch)^2 -------------
            col = 0
            for ch in CONF_CH:
                view = pol[ch].rearrange("(p a) w -> p (a w)", p=128)
                for j in range(fsplit):
                    tin = io.tile([128, free], f32, tag="in")
                    nc.sync.dma_start(tin[:], view[:, j * free : (j + 1) * free])
                    sig = io.tile([128, free], f32, tag="sig")
                    nc.scalar.activation(sig[:], tin[:], ACT_F.Sigmoid)
                    nc.scalar.activation(
                        tin[:], sig[:], ACT_F.Square,
                        accum_out=ACC[:, col : col + 1],
                    )
                    col += 1

            RED = sp.tile([128, 1], f32)
            nc.vector.tensor_reduce(
                out=RED[:], in_=ACC[:], axis=mybir.AxisListType.X, op=ALU.add
            )
            nc.sync.dma_start(out[:], RED[:])

    nc.compile()
    return nc


def get_program():
    global _PROG
    if _PROG is None:
        _PROG = _build_program()
    return _PROG


def make_in_maps(policy_output, target_boxes, target_probs):
    policy_output = np.ascontiguousarray(np.asarray(policy_output, dtype=np.float32))
    target_boxes = np.ascontiguousarray(np.asarray(target_boxes, dtype=np.int32))
    target_probs = np.ascontiguousarray(np.asarray(target_probs, dtype=np.float32))
    assert policy_output.shape == (B, C, H, W)
    in_maps = []
    for i in range(N_CORES):
        in_maps.append(
            {
                "pol": policy_output[i],
                "tb": target_boxes[i],
                "tp": target_probs[i].reshape(T, 1),
                "tri": TRI_CONST,
                "choff": CHOFF_CONST,
            }
        )
    return in_maps


def kernel(policy_output, target_boxes, target_probs):
    from concourse.bass_utils import run_bass_kernel_spmd

    nc = get_program()
    in_maps = make_in_maps(policy_output, target_boxes, target_probs)
    res = run_bass_kernel_spmd(nc, in_maps, list(range(N_CORES)))
    total = 0.0
    for i in range(N_CORES):
        total += float(res.results[i]["out"].sum(dtype=np.float64))
    return np.float32(total / DENOM)
